# revision 1
# baseline (speedup 1.0000x reference)
"""GAE (generalized advantage estimation) kernel for trn2, 8 NeuronCores.

Computes advantages[t] = delta[t] + gl * advantages[t+1] (reverse scan over
T-1=1023 steps) for deltas = rewards[:-1] + gamma*values[1:] - values[:-1],
for 32768 independent batch columns, data-parallel over 8 cores.

Formulation per core (R, V in [1024, 4096] f32 -> A [1023, 4096] f32):
    out[g] = sum_{j>=g} gl^(j-g) * t[j]  +  gamma * sum_{k>g} gl^(k-g-1) * V[k]
with t = R - V. Blocked into 8 time-blocks of 128 rows; each block is two
128x128 matmuls into PSUM (triangular L1 against t, strictly-triangular L2
against V) plus a rank-1 cross-block carry folded into row 0 of the second
matmul (L2 row 0 holds the carry coefficients gl^(128-i); V row 0 is
overwritten with the carry H after its original value is saved).
Carry chain: H_m = psum_m[0] + (gamma/gl) * V_m[0], chained m = 7 -> 0.
"""
import numpy as np

GAMMA = 0.99
LAM = 0.95
GL = GAMMA * LAM
T = 1024
B = 32768
NCORES = 8
BC = B // NCORES          # 4096 batch cols per core
P = 128                   # partitions / time-block size
NB = T // P               # 8 time blocks
CW = 2048                 # batch chunk width (DMA tile)
NCH = BC // CW            # 2 chunks per core
NW = 512                  # matmul moving width (1 PSUM bank, fp32 max)
NSC = CW // NW            # 4 subcols per chunk


def _make_consts():
    ii = np.arange(P)[:, None]  # out row i
    jj = np.arange(P)[None, :]  # in row j
    # U[i, j] = gl^(j-i) for j >= i
    U = np.where(jj >= ii, GL ** (jj - ii), 0.0)
    L1 = U.T.astype(np.float32)  # lhsT: [K=j, M=i]
    L1z = L1.copy()
    L1z[P - 1, :] = 0.0          # kill t[1023] contribution in block 7
    # U2[i, k] = gamma * gl^(k-i-1) for k > i
    U2 = np.where(jj > ii, GAMMA * GL ** (jj - ii - 1.0), 0.0)
    L2 = U2.T.astype(np.float32)
    # carry row: coefficient of H (stored in V row 0) for out row i
    L2[0, :] = (GL ** (P - np.arange(P))).astype(np.float32)
    return L1, L1z, L2


def _build(reps: int = 1):
    import concourse.bacc as bacc
    import concourse.mybir as mybir
    from concourse.tile import TileContext

    f32 = mybir.dt.float32
    nc = bacc.Bacc("TRN2")
    R = nc.dram_tensor("R", [T, BC], f32, kind="ExternalInput")
    V = nc.dram_tensor("V", [T, BC], f32, kind="ExternalInput")
    L1 = nc.dram_tensor("L1", [P, P], f32, kind="ExternalInput")
    L1z = nc.dram_tensor("L1z", [P, P], f32, kind="ExternalInput")
    L2 = nc.dram_tensor("L2", [P, P], f32, kind="ExternalInput")
    A = nc.dram_tensor("A", [T - 1, BC], f32, kind="ExternalOutput")

    mult = mybir.AluOpType.mult
    add = mybir.AluOpType.add

    with TileContext(nc) as tc:
        with (
            tc.tile_pool(name="cst", bufs=1) as cst,
            tc.tile_pool(name="rp", bufs=4) as rp,
            tc.tile_pool(name="vp", bufs=6) as vp,
            tc.tile_pool(name="tp", bufs=5) as tp,
            tc.tile_pool(name="op", bufs=4) as op,
            tc.tile_pool(name="v0p", bufs=3) as v0p,
            tc.tile_pool(name="ps", bufs=8, space="PSUM") as ps,
        ):
            l1 = cst.tile([P, P], f32, tag="l1")
            l1z = cst.tile([P, P], f32, tag="l1z")
            l2 = cst.tile([P, P], f32, tag="l2")
            nc.sync.dma_start(out=l1[:, :], in_=L1[:, :])
            nc.sync.dma_start(out=l1z[:, :], in_=L1z[:, :])
            nc.sync.dma_start(out=l2[:, :], in_=L2[:, :])

            def one_pass():
                # All load DMAs up front, in consumption order (m = 7 .. 0).
                rt = {}
                vt = {}
                for m in range(NB - 1, -1, -1):
                    for ch in range(NCH):
                        r = rp.tile([P, CW], f32, tag="r")
                        v = vp.tile([P, CW], f32, tag="v")
                        cs = slice(ch * CW, (ch + 1) * CW)
                        nc.sync.dma_start(out=r[:, :], in_=R[m * P:(m + 1) * P, cs])
                        nc.sync.dma_start(out=v[:, :], in_=V[m * P:(m + 1) * P, cs])
                        rt[m, ch] = r
                        vt[m, ch] = v

                # Phase A: t = R - V, save V row 0, zero block-7 carry slot.
                # All of these read V row 0 and so MUST be traced before any
                # carry poke overwrites it (Tile serializes in program order).
                # On GpSimd to keep DVE free for the latency-critical carries.
                tt = {}
                v0t = {}
                for m in range(NB - 1, -1, -1):
                    for ch in range(NCH):
                        r, v = rt[m, ch], vt[m, ch]
                        t = tp.tile([P, CW], f32, tag="t")
                        nc.gpsimd.tensor_sub(t[:, :], r[:, :], v[:, :])
                        v0 = v0p.tile([1, CW], f32, tag="v0")
                        nc.gpsimd.tensor_copy(v0[0:1, :], v[0:1, :])
                        if m == NB - 1:
                            # H_8 = 0: no tail beyond t=1023
                            nc.gpsimd.memset(v[0:1, :], 0.0)
                        tt[m, ch] = t
                        v0t[m, ch] = v0

                # Phase B: carry-chained matmuls, blocks m = 7 .. 0.
                for m in range(NB - 1, -1, -1):
                    lhs1 = l1z if m == NB - 1 else l1
                    for ch in range(NCH):
                        v = vt[m, ch]
                        t = tt[m, ch]
                        v0 = v0t[m, ch]
                        stage = op.tile([P, CW], f32, tag="stage")
                        for sc in range(NSC):
                            fs = slice(sc * NW, (sc + 1) * NW)
                            pt = ps.tile([P, NW], f32, tag="ps")
                            nc.tensor.matmul(pt[:, :], lhs1[:, :], t[:, fs],
                                             start=True, stop=False)
                            nc.tensor.matmul(pt[:, :], l2[:, :], v[:, fs],
                                             start=False, stop=True)
                            if m > 0:
                                # H_m = (gamma/gl) * V_m[0] + psum_m[0],
                                # poked into next block's V row 0.
                                nc.vector.scalar_tensor_tensor(
                                    vt[m - 1, ch][0:1, fs], v0[0:1, fs],
                                    GAMMA / GL, pt[0:1, :], mult, add)
                            nc.vector.tensor_copy(stage[:, fs], pt[:, :])
                        cs = slice(ch * CW, (ch + 1) * CW)
                        if m == NB - 1:
                            nc.scalar.dma_start(out=A[m * P:T - 1, cs],
                                                in_=stage[0:P - 1, :])
                        else:
                            nc.scalar.dma_start(out=A[m * P:(m + 1) * P, cs],
                                                in_=stage[:, :])

            for _ in range(reps):
                one_pass()
    nc.finalize()
    return nc


_NC_CACHE = None


def kernel(rewards: np.ndarray, values: np.ndarray) -> np.ndarray:
    from concourse.bass_utils import run_bass_kernel_spmd

    rewards = np.asarray(rewards)
    values = np.asarray(values)

    global _NC_CACHE
    if _NC_CACHE is None:
        _NC_CACHE = _build()
    nc = _NC_CACHE

    L1, L1z, L2 = _make_consts()
    in_maps = []
    for c in range(NCORES):
        cs = slice(c * BC, (c + 1) * BC)
        in_maps.append({
            "R": np.ascontiguousarray(rewards[:, cs], dtype=np.float32),
            "V": np.ascontiguousarray(values[:, cs], dtype=np.float32),
            "L1": L1, "L1z": L1z, "L2": L2,
        })
    res = run_bass_kernel_spmd(nc, in_maps, core_ids=list(range(NCORES)))
    out = np.empty((T - 1, B), dtype=np.float32)
    for c in range(NCORES):
        out[:, c * BC:(c + 1) * BC] = res.results[c]["A"]
    return out



# revision 22
# speedup vs baseline: 3.0317x; 3.0317x over previous
"""GAE (generalized advantage estimation) kernel for trn2, 8 NeuronCores.

Computes advantages[t] = delta[t] + gl * advantages[t+1] (reverse scan over
T-1=1023 steps) for deltas = rewards[:-1] + gamma*values[1:] - values[:-1],
for 32768 independent batch columns, data-parallel over 8 cores.

Design (bf16 I/O, 127-row time blocks, ~42.9us/core vs 129.9us baseline):
  Per core R, V are [1024, 4096] bf16 -> A [1023, 4096] bf16 (host upcasts
  to f32; inputs are rounded to bf16 on the host - rel err ~4.7e-3 vs the
  2e-2 tolerance). Time axis: 8 blocks of 127 output rows + a 7-row tail
  block. With 127-row blocks every delta in a block needs only V rows from
  the same 128-row input window, so the cross-block carry is exactly the
  next block's output row 0 (the advantage at the block boundary), with no
  gamma/gl correction term:
    out_m[i] = sum_{j=i}^{126} gl^(j-i) delta[127m+j]
             + gl^(127-i) * adv[127(m+1)]
  R tile [128, 4096]: partition 0 = carry slot (engine writes must start at
  partition 0, not 127), poked with the next block's stage row 0;
  partitions 1..127 = R rows 127m..127m+126. V tile = V rows 127m..127m+127.
  Per 1024-wide psum tile (2 banks): psum = W^T V (start) + L1c^T R (stop);
  L1c row 0 carries the gl^(127-i) coefficients. All mmV are issued before
  the mmR so only mmR waits on the carry poke. Stage copies (psum f32 ->
  bf16 SBUF, mandatory: DMA cannot read PSUM) are split DVE/Act; carry
  pokes are bf16 SBUF row copies on DVE (4x perf mode).
  DMA transfers occupy the issuing engine in the cost model, so the three
  DMA-capable queues are balanced: R loads on SP, V loads on Pool (SWDGE),
  stores mostly on Act early (while SP/Pool stream loads) and on SP/Pool
  late. A few warmup matmuls hold the PE p-state at full clock from t=0.
"""
import numpy as np

GAMMA = 0.99
LAM = 0.95
GL = GAMMA * LAM
T = 1024
B = 32768
NCORES = 8
BC = B // NCORES          # 4096 batch cols per core
P = 128                   # partitions
BP = 127                  # output rows per full time block
NB = 8                    # full blocks; block index 8 is the 7-row tail
NW = 512                  # matmul moving width (1 PSUM bank, fp32)
PW = 1024                 # psum/stage-copy granularity (2 banks)
NPT = BC // PW            # 4 psum tiles per block
SW = 2048                 # store width (2 stores per block)
NWARM = 4                 # PE p-state warmup matmuls

# Store-queue assignment for the 16 full-block stores (blocks 8..1, two
# each) and the DVE/Act stage split, tuned against the cost model.
STORE_Q = ["act", "act", "act", "act", "act", "act", "act", "pool", "sp",
           "pool", "sp", "pool", "sp", "pool", "sp", "act"]
STAGE_DVE_PTIS = (0, 3)   # psum tiles staged on DVE; 1, 2 go to Act


def _make_consts():
    # L1c [128, 128] lhsT for R tiles: row 0 = carry coeffs, row 1+j = R data
    L1c = np.zeros((P, P), np.float64)
    for i in range(BP):
        L1c[0, i] = GL ** (BP - i)
    for j in range(BP):
        L1c[1 + j, : j + 1] = GL ** (j - np.arange(j + 1))
    # W [128, 128] lhsT for V tiles
    W = np.zeros((P, P), np.float64)
    for i in range(BP):
        W[i, i] = -1.0
        if i + 1 < BP:
            W[i + 1:BP, i] = GAMMA * (1 - LAM) * GL ** (np.arange(i + 1, BP) - 1 - i)
        W[BP, i] = GAMMA * GL ** (BP - 1 - i)
    # tail block (7 outputs from 8 input rows); R_8 row 0 unused (coeff 0)
    L18 = np.zeros((8, P), np.float64)
    for j in range(7):
        L18[1 + j, : j + 1] = GL ** (j - np.arange(j + 1))
    W8 = np.zeros((8, P), np.float64)
    for i in range(7):
        W8[i, i] = -1.0
        if i + 1 < 7:
            W8[i + 1:7, i] = GAMMA * (1 - LAM) * GL ** (np.arange(i + 1, 7) - 1 - i)
        W8[7, i] = GAMMA * GL ** (6 - i)
    from ml_dtypes import bfloat16
    return tuple(np.ascontiguousarray(x, dtype=np.float32).astype(bfloat16)
                 for x in (L1c, W, L18, W8))


def _build(reps: int = 1):
    import concourse.bacc as bacc
    import concourse.mybir as mybir
    from concourse.tile import TileContext

    f32 = mybir.dt.float32
    bf16 = mybir.dt.bfloat16
    nc = bacc.Bacc("TRN2")
    R = nc.dram_tensor("R", [T, BC], bf16, kind="ExternalInput")
    V = nc.dram_tensor("V", [T, BC], bf16, kind="ExternalInput")
    CBIG = nc.dram_tensor("CBIG", [P, 2 * P], bf16, kind="ExternalInput")
    CSML = nc.dram_tensor("CSML", [8, 2 * P], bf16, kind="ExternalInput")
    A = nc.dram_tensor("A", [T - 1, BC], bf16, kind="ExternalOutput")

    store_eng = {"sp": "sync", "act": "scalar", "pool": "gpsimd"}

    with TileContext(nc) as tc:
        with (
            tc.tile_pool(name="cst", bufs=1) as cst,
            tc.tile_pool(name="rp", bufs=4) as rp,
            tc.tile_pool(name="vp", bufs=4) as vp,
            tc.tile_pool(name="stp", bufs=4) as stp,
            tc.tile_pool(name="ps", bufs=4, space="PSUM") as ps,
        ):
            cbig = cst.tile([P, 2 * P], bf16, tag="cbig")
            csml = cst.tile([8, 2 * P], bf16, tag="csml")
            nc.scalar.dma_start(out=csml[:, :], in_=CSML[:, :])
            nc.scalar.dma_start(out=cbig[:, :], in_=CBIG[:, :])
            l1c = cbig[:, 0:P]
            w = cbig[:, P:2 * P]
            l18 = csml[:, 0:P]
            w8 = csml[:, P:2 * P]

            # PE p-state warmup: keep the tensor engine clocked up from t=0
            warm = cst.tile([P, NW], bf16, tag="warm")
            nc.gpsimd.memset(warm[:, :], 0.0)
            wpt = ps.tile([P, PW], f32, tag="ps")
            for _ in range(NWARM):
                nc.tensor.matmul(wpt[:, 0:NW], warm[:, 0:P], warm[:, :],
                                 start=True, stop=True)

            def one_pass():
                # All load DMAs up front in consumption order (m = 8 .. 0).
                # R loads on the SP queue, V loads on the Pool (SWDGE) queue.
                rt, vt = {}, {}
                # tail block loads 1024-wide so the first matmul starts early
                r = rp.tile([8, BC], bf16, tag="r8")
                v = vp.tile([8, BC], bf16, tag="v8")
                for q in range(4):
                    h = slice(q * (BC // 4), (q + 1) * (BC // 4))
                    nc.sync.dma_start(out=r[:, h], in_=R[1015:1023, h])
                    nc.gpsimd.dma_start(out=v[:, h], in_=V[1016:1024, h])
                rt[8], vt[8] = r, v

                for m in range(NB - 1, -1, -1):
                    r = rp.tile([P, BC], bf16, tag="r")
                    v = vp.tile([P, BC], bf16, tag="v")
                    # partitions 1..127 = R rows 127m..127m+126; partition 0
                    # is the carry slot (written by the poke)
                    for h in (slice(0, BC // 2), slice(BC // 2, BC)):
                        nc.sync.dma_start(out=r[1:P, h],
                                          in_=R[BP * m:BP * m + BP, h])
                        nc.gpsimd.dma_start(out=v[:, h],
                                            in_=V[BP * m:BP * m + P, h])
                    rt[m], vt[m] = r, v

                # Blocks m = 8 .. 0; carry = stage row 0 poked into the next
                # (lower) block's R-tile partition 0, per 1024-wide window.
                ncopy = 0
                nstore = 0
                for m in range(NB, -1, -1):
                    lr = l18 if m == NB else l1c
                    lv = w8 if m == NB else w
                    r, v = rt[m], vt[m]
                    st = stp.tile([P, BC], bf16, tag="st")
                    nrow = 7 if m == NB else BP
                    base = 1016 if m == NB else BP * m
                    pts = []
                    # all mmV first: carry-independent PE work
                    for pti in range(NPT):
                        pt = ps.tile([P, PW], f32, tag="ps")
                        pts.append(pt)
                        for sc in range(2):
                            fs = slice(pti * PW + sc * NW,
                                       pti * PW + (sc + 1) * NW)
                            nc.tensor.matmul(pt[:, sc * NW:(sc + 1) * NW],
                                             lv, v[:, fs],
                                             start=True, stop=False)
                    for pti in range(NPT):
                        pt = pts[pti]
                        ws = slice(pti * PW, (pti + 1) * PW)
                        for sc in range(2):
                            fs = slice(pti * PW + sc * NW,
                                       pti * PW + (sc + 1) * NW)
                            nc.tensor.matmul(pt[:, sc * NW:(sc + 1) * NW],
                                             lr, r[:, fs],
                                             start=False, stop=True)
                        # psum->bf16 stage copy (1024 wide), split DVE/Act
                        ncopy += 1
                        if m == 0:
                            # final block: alternate engines per psum tile
                            # and store immediately to shorten the tail
                            if pti % 2 == 0:
                                nc.vector.tensor_copy(st[:, ws], pt[:, :])
                            else:
                                nc.scalar.copy(st[:, ws], pt[:, :])
                            eng = getattr(nc, store_eng[
                                ("sp", "pool", "act", "sp")[pti]])
                            eng.dma_start(out=A[0:BP, ws], in_=st[0:BP, ws])
                        elif pti in STAGE_DVE_PTIS:
                            nc.vector.tensor_copy(st[:, ws], pt[:, :])
                        else:
                            nc.scalar.copy(st[:, ws], pt[:, :])
                        if m > 0:
                            # carry poke: bf16 sbuf row copy (DVE 4x mode)
                            nc.vector.tensor_copy(rt[m - 1][0:1, ws],
                                                  st[0:1, ws])
                    if m != 0:
                        for si in range(BC // SW):
                            cs = slice(si * SW, (si + 1) * SW)
                            eng = getattr(
                                nc, store_eng[STORE_Q[nstore % len(STORE_Q)]])
                            nstore += 1
                            eng.dma_start(out=A[base:base + nrow, cs],
                                          in_=st[0:nrow, cs])

            for _ in range(reps):
                one_pass()
    nc.finalize()
    return nc


_NC_CACHE = None


def kernel(rewards: np.ndarray, values: np.ndarray) -> np.ndarray:
    from ml_dtypes import bfloat16
    from concourse.bass_utils import run_bass_kernel_spmd

    rewards = np.asarray(rewards)
    values = np.asarray(values)

    global _NC_CACHE
    if _NC_CACHE is None:
        _NC_CACHE = _build()
    nc = _NC_CACHE

    L1c, W, L18, W8 = _make_consts()
    cbig = np.ascontiguousarray(np.concatenate([L1c, W], axis=1))
    csml = np.ascontiguousarray(np.concatenate([L18, W8], axis=1))
    in_maps = []
    for c in range(NCORES):
        cs = slice(c * BC, (c + 1) * BC)
        in_maps.append({
            "R": np.ascontiguousarray(rewards[:, cs], dtype=np.float32).astype(bfloat16),
            "V": np.ascontiguousarray(values[:, cs], dtype=np.float32).astype(bfloat16),
            "CBIG": cbig, "CSML": csml,
        })
    res = run_bass_kernel_spmd(nc, in_maps, core_ids=list(range(NCORES)))
    out = np.empty((T - 1, B), dtype=np.float32)
    for c in range(NCORES):
        out[:, c * BC:(c + 1) * BC] = res.results[c]["A"].astype(np.float32)
    return out
